# revision 6
# baseline (speedup 1.0000x reference)
"""Trainium2 Bass kernel for nn_ConvolutionalCapsules.

Sharding: core c (of 8) owns output-capsule nout=c. Each core runs the p4 group
conv restricted to its 64 output channels (16 dout x 4 rot) over all 32 images
(B*Nin), then LayerNorm + degree-score routing + squash for its nout.

Conv: 3x3 conv as shifted matmuls from a zero-padded SBUF image (34x34 rows).
Partitions 0-63 hold the padded image (copy A), partitions 64-127 hold the same
image shifted by one padded row (copy B), so one K=128 matmul covers two filter
taps: (0,kx) on A plus (1,kx) on B at base offset kx. Row-2 taps run as K=64
matmuls on copy A. 6 matmuls accumulate one PSUM tile of 512 positions.

Routing runs in a transposed layout (positions on partitions, (i,d,g) on the
free axis) produced by PE transpose-mode, so every reduction (d, i, g) is a
free-axis tensor_reduce.
"""

import numpy as np
from contextlib import ExitStack

import concourse.bass as bass
import concourse.tile as tile
from concourse import mybir
from concourse.bass_utils import run_bass_kernel_spmd

F32 = mybir.dt.float32
F32R = mybir.dt.float32r
AF = mybir.ActivationFunctionType
OP = mybir.AluOpType
AX = mybir.AxisListType

MM_DT = F32R  # float32r: full-rate PE at ~tf32 precision; set F32 for exact

_ENGINES = {
    mybir.EngineType.PE,
    mybir.EngineType.Activation,
    mybir.EngineType.Pool,
    mybir.EngineType.DVE,
    mybir.EngineType.SP,
}


def _split_sync_waits(nc):
    """This walrus build accepts a single embedded sync-wait per instruction;
    hoist extras onto preceding NoOps on the same engine (ge-imm waits commute)."""
    for f in nc.m.functions:
        for bb in f.blocks:
            newl = []
            changed = False
            for inst in list(bb.instructions):
                si = inst.sync_info
                waits = list(si.on_wait) if si and si.on_wait else []
                if len(waits) > 1 and inst.engine in _ENGINES:
                    changed = True
                    for k, w in enumerate(waits[:-1]):
                        newl.append(
                            mybir.InstNoOp(
                                name=f"{inst.name}-ws{k}",
                                ins=[],
                                outs=[],
                                engine=inst.engine,
                                sync_info=mybir.SyncInfo(on_wait=[w], on_update=[]),
                            )
                        )
                    si.on_wait = waits[-1:]
                    inst.sync_info = si
                newl.append(inst)
            if changed:
                bb.instructions = newl


def build_program(apply_bias=False, apply_gb=False):
    nc = bass.Bass(trn_type="TRN2")
    caps = nc.dram_tensor("caps", [4, 8, 16, 4, 32, 32], MM_DT, kind="ExternalInput")
    w = nc.dram_tensor("w", [128, 384], MM_DT, kind="ExternalInput")
    ident = nc.dram_tensor("ident", [128, 128], F32, kind="ExternalInput")
    zer = nc.dram_tensor("zer", [1, 1164], MM_DT, kind="ExternalInput")
    if apply_bias:
        cb = nc.dram_tensor("cb", [64, 1], F32, kind="ExternalInput")
    if apply_gb:
        gam = nc.dram_tensor("gam", [1, 16], F32, kind="ExternalInput")
        bet = nc.dram_tensor("bet", [1, 16], F32, kind="ExternalInput")
    out = nc.dram_tensor("out", [4, 16, 4, 32, 32], F32, kind="ExternalOutput")

    caps_r = caps.ap().rearrange("b n d g h w -> (b n) (d g) h w")  # [32,64,32,32]
    out_r = out.ap().rearrange("b d g h w -> b (h w) d g")  # [4,1024,16,4]

    XW = 1164  # padded 34x34 image (1156) + slack so 16-row AP views stay in-bounds

    with tile.TileContext(nc) as tc:
        with ExitStack() as ctx:
            singles = ctx.enter_context(tc.tile_pool(name="singles", bufs=1))
            ps_conv = ctx.enter_context(tc.tile_pool(name="ps_conv", bufs=4, space="PSUM"))
            ps_tr = ctx.enter_context(tc.tile_pool(name="ps_tr", bufs=2, space="PSUM"))
            tpool = ctx.enter_context(tc.tile_pool(name="tpool", bufs=2))
            rbig = ctx.enter_context(tc.tile_pool(name="rbig", bufs=2))
            sm = ctx.enter_context(tc.tile_pool(name="sm", bufs=2))
            vout = ctx.enter_context(tc.tile_pool(name="vout", bufs=2))

            w_sb = singles.tile([128, 384], MM_DT, tag="w")
            nc.sync.dma_start(out=w_sb[:], in_=w.ap())
            id_sb = singles.tile([128, 128], F32, tag="ident")
            nc.sync.dma_start(out=id_sb[:], in_=ident.ap())
            if apply_bias:
                cb_sb = singles.tile([64, 1], F32, tag="cb")
                nc.sync.dma_start(out=cb_sb[:], in_=cb.ap())
            if apply_gb:
                gam_sb = singles.tile([128, 16], F32, tag="gam")
                nc.sync.dma_start(out=gam_sb[:], in_=gam.ap().partition_broadcast(128))
                bet_sb = singles.tile([128, 16], F32, tag="bet")
                nc.sync.dma_start(out=bet_sb[:], in_=bet.ap().partition_broadcast(128))

            eps5 = singles.tile([128, 1], F32, tag="eps5")
            nc.vector.memset(eps5[:], 1e-5)
            eps16 = singles.tile([128, 1], F32, tag="eps16")
            nc.vector.memset(eps16[:], 1e-16)

            xpads = []
            for ix in range(3):
                xp = singles.tile([128, XW], MM_DT, tag=f"xpad{ix}", name=f"xpad{ix}")
                nc.sync.dma_start(out=xp[:], in_=zer.ap().partition_broadcast(128))
                xpads.append(xp)

            u_sb = [
                [singles.tile([128, 1024], F32, tag=f"u{b}_{p}", name=f"u{b}_{p}") for p in range(4)]
                for b in range(4)
            ]

            def hview(ap_flat, o, rows):
                """[P, rows, 32] window at flat offset o with padded row stride 34."""
                return ap_flat[:, o: o + rows * 34].rearrange(
                    "c (h w) -> c h w", w=34
                )[:, :, 0:32]

            for b in range(4):
                # ---- conv for the 8 images of this batch ----
                for n in range(8):
                    img = b * 8 + n
                    xp = xpads[img % 3]
                    src = caps_r[img]  # [64,32,32]
                    dstA = hview(xp[0:64], 35, 32)
                    dstB = hview(xp[64:128], 1, 32)
                    nc.sync.dma_start(out=dstA, in_=src)
                    nc.sync.dma_start(out=dstB, in_=src)
                    pair, half = n // 2, n % 2
                    for chh in range(2):
                        ps = ps_conv.tile([64, 512], F32, tag="ps")
                        base = chh * 16 * 34
                        for kx in range(3):
                            rhs = hview(xp, base + kx, 16)
                            nc.tensor.matmul(
                                ps[:],
                                lhsT=w_sb[:, kx * 64:(kx + 1) * 64],
                                rhs=rhs,
                                start=(kx == 0),
                                stop=False,
                            )
                        for kx in range(3):
                            rhs = hview(xp[0:64], base + 68 + kx, 16)
                            nc.tensor.matmul(
                                ps[:],
                                lhsT=w_sb[0:64, (3 + kx) * 64:(4 + kx) * 64],
                                rhs=rhs,
                                start=False,
                                stop=(kx == 2),
                            )
                        dst = u_sb[b][pair][half * 64:(half + 1) * 64, chh * 512:(chh + 1) * 512]
                        if apply_bias:
                            nc.scalar.activation(dst, ps[:], AF.Identity, bias=cb_sb[:], scale=1.0)
                        else:
                            nc.scalar.activation(dst, ps[:], AF.Copy)

                # ---- transpose + routing, two steps of 4 position-blocks ----
                for sh in range(2):
                    T = tpool.tile([128, 2048], F32, tag="T")
                    for bq in range(4):
                        blk = sh * 4 + bq
                        pst = ps_tr.tile([128, 512], F32, tag="pst")
                        for p in range(4):
                            nc.tensor.transpose(
                                out=pst[:, p * 128:(p + 1) * 128],
                                in_=u_sb[b][p][:, blk * 128:(blk + 1) * 128],
                                identity=id_sb[:],
                            )
                        nc.scalar.activation(T[:, bq * 512:(bq + 1) * 512], pst[:], AF.Copy)

                    # views: col = k*512 + i*64 + d*4 + g
                    T5 = T.rearrange("p (k i d g) -> p k i d g", k=4, i=8, d=16)

                    mu = sm.tile([128, 128], F32, tag="mu")
                    mu4 = mu.rearrange("p (k i g) -> p k i g", k=4, i=8)
                    nc.vector.reduce_sum(mu4, T5.transpose((0, 1, 2, 4, 3)), AX.X)

                    sq = rbig.tile([128, 2048], F32, tag="scratch")
                    nc.scalar.activation(sq[:], T[:], AF.Square)
                    sq5 = sq.rearrange("p (k i d g) -> p k i d g", k=4, i=8, d=16)
                    msq = sm.tile([128, 128], F32, tag="msq")
                    msq4 = msq.rearrange("p (k i g) -> p k i g", k=4, i=8)
                    nc.vector.reduce_sum(msq4, sq5.transpose((0, 1, 2, 4, 3)), AX.X)

                    m1 = sm.tile([128, 128], F32, tag="m1")
                    nc.vector.tensor_scalar_mul(out=m1[:], in0=mu[:], scalar1=1.0 / 16.0)
                    var = sm.tile([128, 128], F32, tag="var")
                    nc.vector.tensor_tensor(out=var[:], in0=m1[:], in1=m1[:], op=OP.mult)
                    nc.vector.scalar_tensor_tensor(
                        out=var[:], in0=msq[:], scalar=1.0 / 16.0, in1=var[:],
                        op0=OP.mult, op1=OP.subtract,
                    )
                    rstd = sm.tile([128, 128], F32, tag="rstd")
                    nc.scalar.activation(rstd[:], var[:], AF.Sqrt, bias=eps5[:])
                    nc.vector.reciprocal(rstd[:], rstd[:])
                    n2 = sm.tile([128, 128], F32, tag="n2")
                    nc.vector.tensor_tensor(out=n2[:], in0=m1[:], in1=rstd[:], op=OP.mult)

                    def bc_kig(t):  # [128,128] (k,i,g) -> [p,k,i,d,g]
                        return (
                            t.rearrange("p (k i g) -> p k i g", k=4, i=8)
                            .unsqueeze(3)
                            .broadcast_to((128, 4, 8, 16, 4))
                        )

                    up = rbig.tile([128, 2048], F32, tag="up")
                    up5 = up.rearrange("p (k i d g) -> p k i d g", k=4, i=8, d=16)
                    nc.vector.tensor_tensor(out=up5, in0=T5, in1=bc_kig(rstd), op=OP.mult)
                    nc.vector.tensor_tensor(out=up5, in0=up5, in1=bc_kig(n2), op=OP.subtract)
                    if apply_gb:
                        gb = gam_sb[:].unsqueeze(1).unsqueeze(2).unsqueeze(4).broadcast_to((128, 4, 8, 16, 4))
                        bb_ = bet_sb[:].unsqueeze(1).unsqueeze(2).unsqueeze(4).broadcast_to((128, 4, 8, 16, 4))
                        nc.vector.tensor_tensor(out=up5, in0=up5, in1=gb, op=OP.mult)
                        nc.vector.tensor_tensor(out=up5, in0=up5, in1=bb_, op=OP.add)

                    S = sm.tile([128, 256], F32, tag="S")
                    S4 = S.rearrange("p (k d g) -> p k d g", k=4, d=16)
                    nc.vector.reduce_sum(S4, up5.transpose((0, 1, 3, 4, 2)), AX.X)

                    P = rbig.tile([128, 2048], F32, tag="scratch")
                    P5 = P.rearrange("p (k i d g) -> p k i d g", k=4, i=8, d=16)
                    S_bc = S4.unsqueeze(2).broadcast_to((128, 4, 8, 16, 4))
                    nc.vector.tensor_tensor(out=P5, in0=up5, in1=S_bc, op=OP.mult)
                    dot = sm.tile([128, 128], F32, tag="dot")
                    dot4 = dot.rearrange("p (k i g) -> p k i g", k=4, i=8)
                    nc.vector.reduce_sum(dot4, P5.transpose((0, 1, 2, 4, 3)), AX.X)

                    ns = sm.tile([128, 128], F32, tag="ns")
                    nc.vector.tensor_tensor(out=ns[:], in0=rstd[:], in1=rstd[:], op=OP.mult)
                    nc.vector.scalar_tensor_tensor(
                        out=ns[:], in0=var[:], scalar=16.0, in1=ns[:],
                        op0=OP.mult, op1=OP.mult,
                    )
                    nc.vector.reciprocal(ns[:], ns[:])
                    rr = sm.tile([128, 128], F32, tag="rr")
                    nc.vector.tensor_tensor(out=rr[:], in0=dot[:], in1=ns[:], op=OP.mult)

                    rr4 = rr.rearrange("p (k i g) -> p k i g", k=4, i=8)
                    mx = sm.tile([128, 16], F32, tag="mx")
                    mx3 = mx.rearrange("p (k g) -> p k g", k=4)
                    nc.vector.reduce_max(mx3, rr4.transpose((0, 1, 3, 2)), AX.X)
                    es = sm.tile([128, 128], F32, tag="es")
                    es4 = es.rearrange("p (k i g) -> p k i g", k=4, i=8)
                    mx_bc = mx3.unsqueeze(2).broadcast_to((128, 4, 8, 4))
                    nc.vector.tensor_tensor(out=es4, in0=rr4, in1=mx_bc, op=OP.subtract)
                    nc.scalar.activation(es[:], es[:], AF.Exp)
                    Z = sm.tile([128, 16], F32, tag="Z")
                    Z3 = Z.rearrange("p (k g) -> p k g", k=4)
                    nc.vector.reduce_sum(Z3, es4.transpose((0, 1, 3, 2)), AX.X)
                    nc.vector.reciprocal(Z[:], Z[:])
                    sc = sm.tile([128, 128], F32, tag="sc")
                    sc4 = sc.rearrange("p (k i g) -> p k i g", k=4, i=8)
                    Z_bc = Z3.unsqueeze(2).broadcast_to((128, 4, 8, 4))
                    nc.vector.tensor_tensor(out=sc4, in0=es4, in1=Z_bc, op=OP.mult)

                    nc.vector.tensor_tensor(out=P5, in0=up5, in1=bc_kig(sc), op=OP.mult)
                    s_t = sm.tile([128, 256], F32, tag="s")
                    s4 = s_t.rearrange("p (k d g) -> p k d g", k=4, d=16)
                    nc.vector.reduce_sum(s4, P5.transpose((0, 1, 3, 4, 2)), AX.X)

                    ssq = sm.tile([128, 256], F32, tag="ssq")
                    nc.scalar.activation(ssq[:], s_t[:], AF.Square)
                    nsq = sm.tile([128, 64], F32, tag="nsq")
                    nsq3 = nsq.rearrange("p (k d) -> p k d", k=4)
                    nc.vector.reduce_sum(nsq3, ssq.rearrange("p (k d g) -> p k d g", k=4, d=16), AX.X)
                    sq1 = sm.tile([128, 64], F32, tag="sq1")
                    nc.scalar.activation(sq1[:], nsq[:], AF.Sqrt, bias=eps16[:])
                    nc.vector.scalar_tensor_tensor(
                        out=sq1[:], in0=nsq[:], scalar=1.0, in1=sq1[:],
                        op0=OP.add, op1=OP.mult,
                    )
                    nc.vector.reciprocal(sq1[:], sq1[:])
                    f = sm.tile([128, 64], F32, tag="f")
                    nc.vector.tensor_tensor(out=f[:], in0=nsq[:], in1=sq1[:], op=OP.mult)

                    v = vout.tile([128, 256], F32, tag="v")
                    v4 = v.rearrange("p (k d g) -> p k d g", k=4, d=16)
                    f_bc = f.rearrange("p (k d) -> p k d", k=4).unsqueeze(3).broadcast_to((128, 4, 16, 4))
                    nc.vector.tensor_tensor(out=v4, in0=s4, in1=f_bc, op=OP.mult)

                    dstv = out_r[b].rearrange("(kk p) d g -> p kk d g", p=128)
                    for kk in range(4):
                        nc.sync.dma_start(
                            out=dstv[:, sh * 4 + kk, :, :], in_=v4[:, kk, :, :]
                        )

    _split_sync_waits(nc)
    return nc


def _pack_weights(conv_w):
    w = np.asarray(conv_w, np.float32)
    wt = np.stack(
        [np.roll(np.rot90(w, k=r, axes=(3, 4)), r, axis=2) for r in range(4)], axis=1
    )
    W512 = np.ascontiguousarray(wt.reshape(512, 64, 3, 3), dtype=np.float32)
    packs = []
    for c in range(8):
        Wc = W512[64 * c: 64 * c + 64]
        w_pack = np.zeros((128, 6, 64), np.float32)
        for kx in range(3):
            w_pack[0:64, kx] = Wc[:, :, 0, kx].T
            w_pack[64:128, kx] = Wc[:, :, 1, kx].T
            w_pack[0:64, 3 + kx] = Wc[:, :, 2, kx].T
        packs.append(np.ascontiguousarray(w_pack.reshape(128, 384)))
    return packs


_CACHE = {}


def kernel(capsules, conv_w, conv_b, ln_gamma, ln_beta):
    capsules = np.ascontiguousarray(np.asarray(capsules, np.float32))
    conv_b = np.asarray(conv_b, np.float32)
    ln_gamma = np.asarray(ln_gamma, np.float32)
    ln_beta = np.asarray(ln_beta, np.float32)
    apply_bias = bool(np.any(conv_b))
    apply_gb = bool(np.any(ln_gamma != 1.0) or np.any(ln_beta != 0.0))

    key = (apply_bias, apply_gb)
    if key not in _CACHE:
        _CACHE[key] = build_program(apply_bias=apply_bias, apply_gb=apply_gb)
    nc = _CACHE[key]

    packs = _pack_weights(conv_w)
    ident = np.eye(128, dtype=np.float32)
    in_maps = []
    for c in range(8):
        m = {"caps": capsules, "w": packs[c], "ident": ident,
             "zer": np.zeros((1, 1164), np.float32)}
        if apply_bias:
            b_loc = np.repeat(conv_b[c * 16:(c + 1) * 16], 4)  # partition = d*4+g
            m["cb"] = np.ascontiguousarray(b_loc.reshape(64, 1))
        if apply_gb:
            m["gam"] = np.ascontiguousarray(ln_gamma.reshape(1, 16))
            m["bet"] = np.ascontiguousarray(ln_beta.reshape(1, 16))
        in_maps.append(m)

    res = run_bass_kernel_spmd(nc, in_maps, core_ids=list(range(8)), trace=False)
    out = np.stack([res.results[c]["out"] for c in range(8)], axis=1)
    return out.astype(np.float32)
